# revision 11
# baseline (speedup 1.0000x reference)
"""Trainium2 Bass kernel for nn_HardwareOptimizedSpikeProcessor.

Reference semantics (per timestep t):
    acc += (s_t @ (W*mask).T) * 2**scale_exp     # [B, Cout]
    spk  = acc >= 2**threshold_exp
    acc  = acc * (1 - spk)
    out[:, :, t] = spk

Strategy (~135us HW, vs 184.6us for the batch-sharded 3-op-scan version;
bit-exact vs the fp32 reference):
  - Shard batch/2 x cout/4: each of the 8 cores handles 32 samples x 512
    output channels.  PE work per core is unchanged (8.6 GFLOP bf16 ->
    ~112us, the roofline), but the matmul free dim is b*tb = 32*8 = 256 at a
    t-block of only 8 steps: LDWEIGHTS (~104ns) stays hidden under the
    256-cycle stream while the sequential scan pipelines against the PE at
    8-step granularity -- the post-matmul scan tail shrinks from ~60us to
    ~6us.
  - Matmul is exact: spikes are 0/1 (shipped as fp8e4, mixed with bf16
    weights -- verified exact and full-speed on HW), masked weights are ints
    in [-127,127] exact in bf16, PSUM accumulates fp32 (sums < 2^24).
  - Scan step is 2 DVE instructions instead of 3 (~654ns/step):
        u_t  = acc + c_t                         (tensor_tensor add)
        acc  = (u_t < thr) * u_t                 (scalar_tensor_tensor)
    and spikes (u_t >= thr) are extracted in 4-step batches on the DVE, off
    the serial chain (NOT the Pool engine: its tensor_scalar measures ~10x
    slower than nominal).  The last block's adds read c straight from PSUM,
    skipping the final ACT drain latency.
  - PE p-state warm-up: ~4.8us of junk matmuls bridge the framework preamble
    (~7us) + first DMAs (~13us), so real matmuls start at full clock.
"""

import sys

for _p in ("/opt/trn_rl_repo",):
    if _p not in sys.path:
        sys.path.insert(0, _p)

import numpy as np
import ml_dtypes

import concourse.bass as bass
import concourse.mybir as mybir
import concourse.tile as tile
from concourse.bass_utils import run_bass_kernel_spmd

B, CIN, COUT, T = 64, 2048, 2048, 128
NCORES = 8
NB = 2                      # batch shards
NQ = 4                      # cout shards
BLOC = B // NB              # 32 samples per core
QLOC = COUT // NQ           # 512 output channels per core
MC = QLOC // 128            # 4 output-channel chunks per core
KC = CIN // 128             # 16 contraction chunks

# scan blocks along T: uniform 8-step blocks pipeline the serial scan
# against the PE with the smallest post-matmul tail.  (16-step mid blocks
# give ~1% better PE cadence but let the scan lag build up into a larger
# tail -- measured net loss.)
BLOCKS = [(8 * i, 8) for i in range(16)]
TBMAX = 8
# spike DMA chunks along T (first small so PE starts early; every block
# falls entirely within one chunk)
TCHUNKS = [8, 8, 16, 32, 32, 32]
NBLK = len(BLOCKS)
assert sum(TCHUNKS) == T
assert sum(tb for _, tb in BLOCKS) == T

NJUNK = 8                   # p-state warm-up matmuls (~1.7us at mid clock)

_MAX_WAITS = 1


def _split_excess_waits(nc):
    """This container's walrus build accepts at most one sync-wait per
    instruction; spill extra waits onto same-engine NOPs placed before the
    offending instruction."""
    for f in nc.m.functions:
        for bb in f.blocks:
            new_list = []
            for ins in bb.instructions:
                si = ins.sync_info
                waits = list(si.on_wait) if si is not None and si.on_wait else []
                if len(waits) > _MAX_WAITS:
                    extra, keep = waits[:-_MAX_WAITS], waits[-_MAX_WAITS:]
                    for i in range(0, len(extra), _MAX_WAITS):
                        nop = mybir.InstNoOp(
                            name=f"{ins.name}-waitsplit-{i}", ins=[], outs=[]
                        )
                        nop.engine = ins.engine
                        nop.sync_info = mybir.SyncInfo(
                            on_wait=extra[i : i + _MAX_WAITS], on_update=[]
                        )
                        new_list.append(nop)
                    ins.sync_info = mybir.SyncInfo(
                        on_wait=keep,
                        on_update=list(si.on_update) if si.on_update else [],
                    )
                new_list.append(ins)
            bb.instructions[:] = new_list


def _build(thr: float):
    f32 = mybir.dt.float32
    bf16 = mybir.dt.bfloat16
    fp8 = mybir.dt.float8e4
    u8 = mybir.dt.uint8
    i8 = mybir.dt.int8
    nc = bass.Bass()

    # W^T (2**scale_exp folded in): per m-chunk [cin_lo, k, cout_lo], shipped
    # as int8 (values are ints in [-127,127]) to halve the startup-critical
    # weight DMA; converted to bf16 on the otherwise-idle DVE before use.
    wt_ds = [
        nc.dram_tensor(f"wt{m}", [128, KC, 128], i8, kind="ExternalInput")
        for m in range(MC)
    ]
    # spike chunks, each contiguous [cin_lo, k, b, tc]
    spk_ds = [
        nc.dram_tensor(f"spk{j}", [128, KC, BLOC, tc], fp8, kind="ExternalInput")
        for j, tc in enumerate(TCHUNKS)
    ]
    # per-block spike outputs [cout_lo, t, m, b]
    out_ds = [
        nc.dram_tensor(f"out{j}", [128, tb, MC, BLOC], u8, kind="ExternalOutput")
        for j, (_, tb) in enumerate(BLOCKS)
    ]

    # block -> (chunk index, t offset within chunk)
    cstart = []
    s = 0
    for tc in TCHUNKS:
        cstart.append(s)
        s += tc
    blk_map = []
    for t0, tb in BLOCKS:
        cj = max(i for i, cs in enumerate(cstart) if cs <= t0)
        assert t0 + tb <= cstart[cj] + TCHUNKS[cj]
        blk_map.append((cj, t0 - cstart[cj]))

    with tile.TileContext(nc) as tc:
        with (
            tc.tile_pool(name="const", bufs=1) as const,
            tc.tile_pool(name="cpool", bufs=3) as cpool,
            tc.tile_pool(name="upool", bufs=3) as upool,
            tc.tile_pool(name="opool", bufs=3) as opool,
            tc.tile_pool(name="psum", bufs=3, space="PSUM") as psum,
        ):
            wt_sb = const.tile([128, MC, KC, 128], bf16)
            spk_sbs = [
                const.tile([128, KC, BLOC, tc], fp8, name=f"spk_sb{j}")
                for j, tc in enumerate(TCHUNKS)
            ]
            acc = const.tile([128, MC, BLOC], f32)
            junk = const.tile([128, 256], bf16)

            # Weights ship as int8 and are cast to bf16 by the gpsimd
            # software-DGE DMA in flight (halves the startup-critical HBM
            # bytes, no on-chip convert pass); the gpsimd engine body also
            # starts ~1.2us before the sync engine's.  Spike chunks ride the
            # sync HWDGE queue, interleaved with the weight demand.
            nc.gpsimd.memset(junk[:], 0.0)
            nc.gpsimd.dma_start(wt_sb[:, 0], wt_ds[0][:])
            nc.gpsimd.dma_start(wt_sb[:, 1], wt_ds[1][:])
            nc.gpsimd.dma_start(wt_sb[:, 2], wt_ds[2][:])
            nc.gpsimd.dma_start(wt_sb[:, 3], wt_ds[3][:])
            nc.sync.dma_start(spk_sbs[0][:, :8], spk_ds[0][:, :8])
            nc.sync.dma_start(spk_sbs[0][:, 8:], spk_ds[0][:, 8:])
            nc.sync.dma_start(spk_sbs[1][:], spk_ds[1][:])
            nc.vector.memset(acc[:], 0.0)
            # block index at which to issue each remaining chunk's DMA
            # (chunk c's first block minus 2)
            chunk_issue = {}
            for cidx in range(2, len(TCHUNKS)):
                first_blk = next(
                    jj for jj, (cj2, _) in enumerate(blk_map) if cj2 == cidx
                )
                chunk_issue.setdefault(max(0, first_blk - 2), []).append(cidx)

            # PE p-state warm-up on junk data while the DMAs land; the warm-up
            # psum borrows a slot of the main psum ring (freed before block 1
            # needs it)
            wps = psum.tile([128, MC, BLOC * TBMAX], f32, tag="ps", name="ps")
            for _ in range(NJUNK):
                nc.tensor.matmul(wps[:, 0, :256], lhsT=junk[:, :128], rhs=junk[:])

            for j, (t0, tb) in enumerate(BLOCKS):
                for cidx in chunk_issue.get(j, []):
                    nc.sync.dma_start(spk_sbs[cidx][:], spk_ds[cidx][:])
                cj, toff = blk_map[j]
                nfree = BLOC * tb
                ps = psum.tile([128, MC, BLOC * TBMAX], f32, tag="ps", name="ps")
                for m in range(MC):
                    for k in range(KC):
                        nc.tensor.matmul(
                            ps[:, m, :nfree],
                            lhsT=wt_sb[:, m, k, :],
                            rhs=spk_sbs[cj][:, k, :, toff : toff + tb],
                            start=(k == 0),
                            stop=(k == KC - 1),
                        )
                # PSUM [p, m, (b t)] -> SBUF c [p, t, m, b] so each scan step
                # reads a contiguous [128, (m b)] slice.  Drained in 8-step
                # granules so the scan starts while later granules drain.
                last = j == NBLK - 1
                ps_v = ps[:, :, :nfree].rearrange("p m (b t) -> p m b t", b=BLOC)
                H = tb // 2
                ngr = tb // H
                if not last:
                    c = cpool.tile([128, TBMAX, MC, BLOC], f32, tag="cblk")
                    for h in range(ngr):
                        nc.scalar.copy(
                            c[:, h * H : (h + 1) * H].rearrange(
                                "p t m b -> p m b t"
                            ),
                            ps_v[:, :, :, h * H : (h + 1) * H],
                        )
                u = upool.tile([128, TBMAX, MC, BLOC], f32, tag="ublk")
                ob = opool.tile([128, TBMAX, MC, BLOC], u8, tag="oblk")
                for h in range(ngr):
                    for t in range(h * H, (h + 1) * H):
                        # last block: read c straight from PSUM (skips the
                        # ACT drain latency right at the kernel tail)
                        c_t = ps_v[:, :, :, t] if last else c[:, t]
                        nc.vector.tensor_tensor(
                            u[:, t], acc[:], c_t, mybir.AluOpType.add
                        )
                        # the reset after the very last timestep is dead code
                        # (acc is never read again) -- skip it off the tail
                        if not (last and t == tb - 1):
                            nc.vector.scalar_tensor_tensor(
                                acc[:], u[:, t], thr, u[:, t],
                                mybir.AluOpType.is_lt, mybir.AluOpType.mult,
                            )
                    # spikes = (u >= thr), per granule off the serial chain
                    # (DVE: the Pool engine runs tensor_scalar ~10x slower
                    # than its nominal rate, measured 15.6us per block)
                    nc.vector.tensor_scalar(
                        ob[:, h * H : (h + 1) * H],
                        u[:, h * H : (h + 1) * H],
                        thr, None, mybir.AluOpType.is_ge,
                    )
                    nc.sync.dma_start(
                        out_ds[j][:, h * H : (h + 1) * H],
                        ob[:, h * H : (h + 1) * H],
                    )

    _split_excess_waits(nc)
    return nc


def _prep_inputs(spikes, weights, mask, scale_exp):
    wm = weights * mask  # integers <= 127, exact
    scale = np.exp2(scale_exp.astype(np.float64)).astype(np.float32)
    wm = (wm * scale[:, None]).astype(np.float32)  # fold power-of-2 scale in
    in_maps = []
    for core in range(NCORES):
        bh, cq = divmod(core, NQ)
        # weights for this cout shard: [qloc, cin] -> W^T -> [m, cin_lo, k, cout_lo]
        wq = wm[cq * QLOC : (cq + 1) * QLOC]  # [512, 2048]
        wt = (
            wq.T.reshape(KC, 128, MC, 128)
            .transpose(2, 1, 0, 3)
            .astype(np.int8)
        )  # [m, cin_lo, k, cout_lo]; ints in [-127,127], exact in int8
        m = {f"wt{mm}": np.ascontiguousarray(wt[mm]) for mm in range(MC)}
        # spikes for this batch shard: [b, cin, t] -> [cin_lo, k, b, t]
        s = spikes[bh * BLOC : (bh + 1) * BLOC]
        a = s.transpose(1, 0, 2).reshape(KC, 128, BLOC, T).transpose(1, 0, 2, 3)
        a = a.astype(ml_dtypes.float8_e4m3)
        t0 = 0
        for jj, tc in enumerate(TCHUNKS):
            m[f"spk{jj}"] = np.ascontiguousarray(a[:, :, :, t0 : t0 + tc])
            t0 += tc
        in_maps.append(m)
    return in_maps


_CACHE = {}


def _get_program(thr: float):
    if thr not in _CACHE:
        _CACHE[thr] = _build(thr)
    return _CACHE[thr]


def kernel(spikes, weights, mask, scale_exp, threshold_exp, **run_kwargs):
    thr = float(2.0 ** int(np.asarray(threshold_exp)))
    nc = _get_program(thr)
    in_maps = _prep_inputs(
        np.asarray(spikes, dtype=np.float32),
        np.asarray(weights, dtype=np.float32),
        np.asarray(mask, dtype=np.float32),
        np.asarray(scale_exp),
    )
    res = run_bass_kernel_spmd(
        nc, in_maps, core_ids=list(range(NCORES)), **run_kwargs
    )
    full = np.zeros((B, COUT, T), dtype=np.float32)
    for core in range(NCORES):
        bh, cq = divmod(core, NQ)
        blks = [
            np.asarray(res.results[core][f"out{j}"]) for j in range(NBLK)
        ]  # each [cout_lo, t, m, b]
        a = np.concatenate(blks, axis=1)  # [cout_lo, T, m, b]
        # -> [b, m, cout_lo, T] -> [b_loc, qloc, T]
        a = a.transpose(3, 2, 0, 1).reshape(BLOC, QLOC, T)
        full[bh * BLOC : (bh + 1) * BLOC, cq * QLOC : (cq + 1) * QLOC] = a
    if run_kwargs:
        return full, res
    return full



# revision 14
# speedup vs baseline: 1.1065x; 1.1065x over previous
"""Trainium2 Bass kernel for nn_HardwareOptimizedSpikeProcessor.

Reference semantics (per timestep t):
    acc += (s_t @ (W*mask).T) * 2**scale_exp     # [B, Cout]
    spk  = acc >= 2**threshold_exp
    acc  = acc * (1 - spk)
    out[:, :, t] = spk

Strategy (~135us HW, vs 184.6us for the batch-sharded 3-op-scan version;
bit-exact vs the fp32 reference):
  - Shard batch/2 x cout/4: each of the 8 cores handles 32 samples x 512
    output channels.  PE work per core is unchanged (8.6 GFLOP bf16 ->
    ~112us, the roofline), but the matmul free dim is b*tb = 32*8 = 256 at a
    t-block of only 8 steps: LDWEIGHTS (~104ns) stays hidden under the
    256-cycle stream while the sequential scan pipelines against the PE at
    8-step granularity -- the post-matmul scan tail shrinks from ~60us to
    ~6us.
  - Matmul is exact: spikes are 0/1 (shipped as fp8e4, mixed with bf16
    weights -- verified exact and full-speed on HW), masked weights are ints
    in [-127,127] exact in bf16, PSUM accumulates fp32 (sums < 2^24).
  - Scan step is 2 DVE instructions instead of 3 (~654ns/step):
        u_t  = acc + c_t                         (tensor_tensor add)
        acc  = (u_t < thr) * u_t                 (scalar_tensor_tensor)
    and spikes (u_t >= thr) are extracted in 4-step batches on the DVE, off
    the serial chain (NOT the Pool engine: its tensor_scalar measures ~10x
    slower than nominal).  The last block's adds read c straight from PSUM,
    skipping the final ACT drain latency.
  - PE p-state warm-up: ~4.8us of junk matmuls bridge the framework preamble
    (~7us) + first DMAs (~13us), so real matmuls start at full clock.
"""

import sys

for _p in ("/opt/trn_rl_repo",):
    if _p not in sys.path:
        sys.path.insert(0, _p)

import numpy as np
import ml_dtypes

import concourse.bass as bass
import concourse.mybir as mybir
import concourse.tile as tile
from concourse.bass_utils import run_bass_kernel_spmd

B, CIN, COUT, T = 64, 2048, 2048, 128
NCORES = 8
NB = 2                      # batch shards
NQ = 4                      # cout shards
BLOC = B // NB              # 32 samples per core
QLOC = COUT // NQ           # 512 output channels per core
MC = QLOC // 128            # 4 output-channel chunks per core
KC = CIN // 128             # 16 contraction chunks

# scan blocks along T: uniform 8-step blocks pipeline the serial scan
# against the PE with the smallest post-matmul tail.  (16-step mid blocks
# give ~1% better PE cadence but let the scan lag build up into a larger
# tail -- measured net loss.)
BLOCKS = [(8 * i, 8) for i in range(16)]
TBMAX = 8
# spike DMA chunks along T (first small so PE starts early; every block
# falls entirely within one chunk)
TCHUNKS = [8, 8, 16, 32, 32, 32]
NBLK = len(BLOCKS)
assert sum(TCHUNKS) == T
assert sum(tb for _, tb in BLOCKS) == T

NJUNK = 15                  # p-state warm-up matmuls (~3.2us at mid clock)

_MAX_WAITS = 1


def _split_excess_waits(nc):
    """This container's walrus build accepts at most one sync-wait per
    instruction; spill extra waits onto same-engine NOPs placed before the
    offending instruction."""
    for f in nc.m.functions:
        for bb in f.blocks:
            new_list = []
            for ins in bb.instructions:
                si = ins.sync_info
                waits = list(si.on_wait) if si is not None and si.on_wait else []
                if len(waits) > _MAX_WAITS:
                    extra, keep = waits[:-_MAX_WAITS], waits[-_MAX_WAITS:]
                    for i in range(0, len(extra), _MAX_WAITS):
                        nop = mybir.InstNoOp(
                            name=f"{ins.name}-waitsplit-{i}", ins=[], outs=[]
                        )
                        nop.engine = ins.engine
                        nop.sync_info = mybir.SyncInfo(
                            on_wait=extra[i : i + _MAX_WAITS], on_update=[]
                        )
                        new_list.append(nop)
                    ins.sync_info = mybir.SyncInfo(
                        on_wait=keep,
                        on_update=list(si.on_update) if si.on_update else [],
                    )
                new_list.append(ins)
            bb.instructions[:] = new_list


def _build(thr: float):
    f32 = mybir.dt.float32
    bf16 = mybir.dt.bfloat16
    fp8 = mybir.dt.float8e4
    u8 = mybir.dt.uint8
    i8 = mybir.dt.int8
    nc = bass.Bass()

    # W^T (2**scale_exp folded in): per m-chunk [cin_lo, k, cout_lo], shipped
    # as int8 (values are ints in [-127,127]) to halve the startup-critical
    # weight DMA; converted to bf16 on the otherwise-idle DVE before use.
    wt_ds = [
        nc.dram_tensor(f"wt{m}", [128, KC, 128], i8, kind="ExternalInput")
        for m in range(MC)
    ]
    # spike chunks, each contiguous [cin_lo, k, b, tc]
    spk_ds = [
        nc.dram_tensor(f"spk{j}", [128, KC, BLOC, tc], fp8, kind="ExternalInput")
        for j, tc in enumerate(TCHUNKS)
    ]
    # per-block spike outputs [cout_lo, t, m, b]
    out_ds = [
        nc.dram_tensor(f"out{j}", [128, tb, MC, BLOC], u8, kind="ExternalOutput")
        for j, (_, tb) in enumerate(BLOCKS)
    ]

    # block -> (chunk index, t offset within chunk)
    cstart = []
    s = 0
    for tc in TCHUNKS:
        cstart.append(s)
        s += tc
    blk_map = []
    for t0, tb in BLOCKS:
        cj = max(i for i, cs in enumerate(cstart) if cs <= t0)
        assert t0 + tb <= cstart[cj] + TCHUNKS[cj]
        blk_map.append((cj, t0 - cstart[cj]))

    with tile.TileContext(nc) as tc:
        with (
            tc.tile_pool(name="const", bufs=1) as const,
            tc.tile_pool(name="cpool", bufs=3) as cpool,
            tc.tile_pool(name="upool", bufs=3) as upool,
            tc.tile_pool(name="opool", bufs=3) as opool,
            tc.tile_pool(name="psum", bufs=3, space="PSUM") as psum,
        ):
            wt_sb = const.tile([128, MC, KC, 128], bf16)
            wt_i8 = const.tile([128, MC, KC, 128], i8)
            spk_sbs = [
                const.tile([128, KC, BLOC, tc], fp8, name=f"spk_sb{j}")
                for j, tc in enumerate(TCHUNKS)
            ]
            acc = const.tile([128, MC, BLOC], f32)
            junk = const.tile([128, 256], bf16)

            # All input DMAs ride the single sync HWDGE queue in demand order
            # (a second HWDGE queue or the gpsimd swdge queue both measured
            # ~100GB/s and stole bandwidth from q1).  Weights ship as int8
            # (halves the startup-critical bytes) in half-m pieces that a
            # DVE convert pipelines to bf16 right behind the DMA stream.
            nc.gpsimd.memset(junk[:], 0.0)
            nc.vector.memset(acc[:], 0.0)
            nc.sync.dma_start(spk_sbs[0][:, :8], spk_ds[0][:, :8])
            for m in range(MC):
                nc.sync.dma_start(wt_i8[:, m, : KC // 2], wt_ds[m][:, : KC // 2])
                nc.sync.dma_start(wt_i8[:, m, KC // 2 :], wt_ds[m][:, KC // 2 :])
                if m == 0:
                    nc.sync.dma_start(spk_sbs[0][:, 8:], spk_ds[0][:, 8:])
            nc.sync.dma_start(spk_sbs[1][:], spk_ds[1][:])
            # int8->bf16 converts on the DVE (idle until the block-0 scan)
            for m in range(MC):
                for h in range(2):
                    sl = slice(h * KC // 2, (h + 1) * KC // 2)
                    nc.vector.tensor_scalar(
                        wt_sb[:, m, sl], wt_i8[:, m, sl], 0.0, None,
                        mybir.AluOpType.add,
                    )
            # block index at which to issue each remaining chunk's DMA
            # (chunk c's first block minus 2)
            chunk_issue = {}
            for cidx in range(2, len(TCHUNKS)):
                first_blk = next(
                    jj for jj, (cj2, _) in enumerate(blk_map) if cj2 == cidx
                )
                chunk_issue.setdefault(max(0, first_blk - 2), []).append(cidx)

            # PE p-state warm-up on junk data while the DMAs land; the warm-up
            # psum borrows a slot of the main psum ring (freed before block 1
            # needs it)
            wps = psum.tile([128, MC, BLOC * TBMAX], f32, tag="ps", name="ps")
            for _ in range(NJUNK):
                nc.tensor.matmul(wps[:, 0, :256], lhsT=junk[:, :128], rhs=junk[:])

            for j, (t0, tb) in enumerate(BLOCKS):
                for cidx in chunk_issue.get(j, []):
                    nc.sync.dma_start(spk_sbs[cidx][:], spk_ds[cidx][:])
                cj, toff = blk_map[j]
                nfree = BLOC * tb
                ps = psum.tile([128, MC, BLOC * TBMAX], f32, tag="ps", name="ps")
                for m in range(MC):
                    for k in range(KC):
                        nc.tensor.matmul(
                            ps[:, m, :nfree],
                            lhsT=wt_sb[:, m, k, :],
                            rhs=spk_sbs[cj][:, k, :, toff : toff + tb],
                            start=(k == 0),
                            stop=(k == KC - 1),
                        )
                # PSUM [p, m, (b t)] -> SBUF c [p, t, m, b] so each scan step
                # reads a contiguous [128, (m b)] slice.  Drained in 8-step
                # granules so the scan starts while later granules drain.
                last = j == NBLK - 1
                ps_v = ps[:, :, :nfree].rearrange("p m (b t) -> p m b t", b=BLOC)
                H = tb // 2
                ngr = tb // H
                if not last:
                    c = cpool.tile([128, TBMAX, MC, BLOC], f32, tag="cblk")
                    for h in range(ngr):
                        nc.scalar.copy(
                            c[:, h * H : (h + 1) * H].rearrange(
                                "p t m b -> p m b t"
                            ),
                            ps_v[:, :, :, h * H : (h + 1) * H],
                        )
                u = upool.tile([128, TBMAX, MC, BLOC], f32, tag="ublk")
                ob = opool.tile([128, TBMAX, MC, BLOC], u8, tag="oblk")
                for h in range(ngr):
                    for t in range(h * H, (h + 1) * H):
                        # last block: read c straight from PSUM (skips the
                        # ACT drain latency right at the kernel tail)
                        c_t = ps_v[:, :, :, t] if last else c[:, t]
                        nc.vector.tensor_tensor(
                            u[:, t], acc[:], c_t, mybir.AluOpType.add
                        )
                        # the reset after the very last timestep is dead code
                        # (acc is never read again) -- skip it off the tail
                        if not (last and t == tb - 1):
                            nc.vector.scalar_tensor_tensor(
                                acc[:], u[:, t], thr, u[:, t],
                                mybir.AluOpType.is_lt, mybir.AluOpType.mult,
                            )
                    # spikes = (u >= thr), per granule off the serial chain
                    # (DVE: the Pool engine runs tensor_scalar ~10x slower
                    # than its nominal rate, measured 15.6us per block)
                    nc.vector.tensor_scalar(
                        ob[:, h * H : (h + 1) * H],
                        u[:, h * H : (h + 1) * H],
                        thr, None, mybir.AluOpType.is_ge,
                    )
                    nc.sync.dma_start(
                        out_ds[j][:, h * H : (h + 1) * H],
                        ob[:, h * H : (h + 1) * H],
                    )

    _split_excess_waits(nc)
    return nc


def _prep_inputs(spikes, weights, mask, scale_exp):
    wm = weights * mask  # integers <= 127, exact
    scale = np.exp2(scale_exp.astype(np.float64)).astype(np.float32)
    wm = (wm * scale[:, None]).astype(np.float32)  # fold power-of-2 scale in
    in_maps = []
    for core in range(NCORES):
        bh, cq = divmod(core, NQ)
        # weights for this cout shard: [qloc, cin] -> W^T -> [m, cin_lo, k, cout_lo]
        wq = wm[cq * QLOC : (cq + 1) * QLOC]  # [512, 2048]
        wt = (
            wq.T.reshape(KC, 128, MC, 128)
            .transpose(2, 1, 0, 3)
            .astype(np.int8)
        )  # [m, cin_lo, k, cout_lo]; ints in [-127,127], exact in int8
        m = {f"wt{mm}": np.ascontiguousarray(wt[mm]) for mm in range(MC)}
        # spikes for this batch shard: [b, cin, t] -> [cin_lo, k, b, t]
        s = spikes[bh * BLOC : (bh + 1) * BLOC]
        a = s.transpose(1, 0, 2).reshape(KC, 128, BLOC, T).transpose(1, 0, 2, 3)
        a = a.astype(ml_dtypes.float8_e4m3)
        t0 = 0
        for jj, tc in enumerate(TCHUNKS):
            m[f"spk{jj}"] = np.ascontiguousarray(a[:, :, :, t0 : t0 + tc])
            t0 += tc
        in_maps.append(m)
    return in_maps


_CACHE = {}


def _get_program(thr: float):
    if thr not in _CACHE:
        _CACHE[thr] = _build(thr)
    return _CACHE[thr]


def kernel(spikes, weights, mask, scale_exp, threshold_exp, **run_kwargs):
    thr = float(2.0 ** int(np.asarray(threshold_exp)))
    nc = _get_program(thr)
    in_maps = _prep_inputs(
        np.asarray(spikes, dtype=np.float32),
        np.asarray(weights, dtype=np.float32),
        np.asarray(mask, dtype=np.float32),
        np.asarray(scale_exp),
    )
    res = run_bass_kernel_spmd(
        nc, in_maps, core_ids=list(range(NCORES)), **run_kwargs
    )
    full = np.zeros((B, COUT, T), dtype=np.float32)
    for core in range(NCORES):
        bh, cq = divmod(core, NQ)
        blks = [
            np.asarray(res.results[core][f"out{j}"]) for j in range(NBLK)
        ]  # each [cout_lo, t, m, b]
        a = np.concatenate(blks, axis=1)  # [cout_lo, T, m, b]
        # -> [b, m, cout_lo, T] -> [b_loc, qloc, T]
        a = a.transpose(3, 2, 0, 1).reshape(BLOC, QLOC, T)
        full[bh * BLOC : (bh + 1) * BLOC, cq * QLOC : (cq + 1) * QLOC] = a
    if run_kwargs:
        return full, res
    return full

